# revision 1
# baseline (speedup 1.0000x reference)
"""DirectedHyperConvLayer — Trainium2 Bass kernel (8-core SPMD).

Computes (matching the reference nn.Module):
    msg_tar = segment_sum(tar_vals * pois_embs[tar_cols], tar_rows, 50000)
    msg_src = segment_sum(src_vals * msg_tar[src_cols],  src_rows, 100000)
    out     = dropout(relu(msg_src), p=0.3, key=42)    # inverted dropout

Distribution: output rows of each SpMM are sharded across the 8 cores
(row-parallel SpMM, pois_embs replicated), with an AllGather of msg_tar
between the two SpMMs.  Per core, nnz are sorted by (superblock,
col-chunk, row-block) and padded to 128-nnz tiles that are block- and
chunk-pure.  Dense rows are fetched with gpsimd.dma_gather (int16
chunk-local indices on 4 rotating SWDGE queues); the segment sum runs on
the tensor engine as psum[block] += S^T @ G with the sparse-value matrix
S built on the vector engine (batched is_equal + mult against an iota).
PSUM accumulators are packed two-per-bank (partition halves).  ReLU and
the dropout mask (resident uint8, with the 1/(1-p) factor folded into
the SpMM2 values host-side) are fused into the epilogue.  Outputs use a
partition-major DRAM layout (big DMA descriptors); the host undoes the
permutation.
"""

import sys

if "/opt/trn_rl_repo" not in sys.path:
    sys.path.insert(0, "/opt/trn_rl_repo")

from dataclasses import dataclass

import numpy as np

import concourse.tile as tile
from concourse import bacc, mybir
from concourse.bass_utils import run_bass_kernel_spmd

F32 = mybir.dt.float32
I16 = mybir.dt.int16

# problem constants (hardcoded per contract)
N_POIS = 100000
N_EDGES = 50000
D = 128
DROP_P = 0.3
N_CORES = 8

# kernel tuning
CHUNK = 32768          # int16 index reach per gather table slice
M = 64                 # rows per output block (PSUM partition-half)
SB = 8                 # blocks per superblock (4 PSUM banks x 2 halves)
W = 8                  # tiles per dma_gather call (1024 indices)
NQ = 4                 # SWDGE queues
SCRATCH = 98304        # dynamic-DMA descriptor ring bytes


# ---------------------------------------------------------------- schedule

@dataclass
class BlockInfo:
    b: int
    sb: int
    tile_j: int
    half: int
    first_t: int
    last_t: int
    row0: int
    prows: int


@dataclass
class SpmmSchedule:
    T: int
    nsb: int
    calls: list
    tile_block: list
    blocks: list
    sb_ntiles_j: list
    rows_per_core: int


def build_schedule(tiles_shb, nb, nch, rows_per_core):
    nsb = len(tiles_shb)
    calls = []
    tile_block = []
    first_t = {}
    last_t = {}
    t = 0
    for sb in range(nsb):
        nb_sb = len(tiles_shb[sb][0])
        for h in range(nch):
            run_t0 = t
            for bl in range(nb_sb):
                b = sb * SB + bl
                for _ in range(tiles_shb[sb][h][bl]):
                    tile_block.append(b)
                    if b not in first_t:
                        first_t[b] = t
                    last_t[b] = t
                    t += 1
            o = run_t0
            while o < t:
                w = min(W, t - o)
                calls.append((sb, h, o, w))
                o += w
    T = t
    blocks = []
    for b in range(nb):
        sb = b // SB
        bl = b - sb * SB
        row0 = b * M
        blocks.append(
            BlockInfo(
                b=b, sb=sb, tile_j=bl // 2, half=bl % 2,
                first_t=first_t[b], last_t=last_t[b],
                row0=row0, prows=min(M, rows_per_core - row0),
            )
        )
    sb_ntiles_j = [(len(tiles_shb[sb][0]) + 1) // 2 for sb in range(nsb)]
    return SpmmSchedule(
        T=T, nsb=nsb, calls=calls, tile_block=tile_block, blocks=blocks,
        sb_ntiles_j=sb_ntiles_j, rows_per_core=rows_per_core,
    )


def prep_spmm(rows, cols, vals, R, n_cores, nch, chunk_of, local_of):
    rows = np.asarray(rows).astype(np.int64)
    cols = np.asarray(cols).astype(np.int64)
    vals = np.asarray(vals).astype(np.float32)
    C = n_cores
    rows_per_core = R // C
    nb = -(-rows_per_core // M)
    nsb = -(-nb // SB)

    per_core = []
    counts = np.zeros((C, nsb, nch, nb), np.int64)
    for c in range(C):
        base = c * rows_per_core
        sel = (rows >= base) & (rows < base + rows_per_core)
        lr = rows[sel] - base
        cc = cols[sel]
        vv = vals[sel]
        b = lr // M
        h = chunk_of(cc)
        sbk = b // SB
        order = np.lexsort((b, h, sbk))
        lr, cc, vv, b, h = lr[order], cc[order], vv[order], b[order], h[order]
        key = (b // SB * nch + h) * nb + b
        np.add.at(counts[c].reshape(-1), key, 1)
        per_core.append((lr, cc, vv))

    mx = counts.max(axis=0)
    tiles_shb = []
    for sb in range(nsb):
        nb_sb = min(SB, nb - sb * SB)
        tiles_shb.append(
            [
                [-(-int(mx[sb, h, sb * SB + bl]) // 128) for bl in range(nb_sb)]
                for h in range(nch)
            ]
        )
    for sb in range(nsb):
        nb_sb = min(SB, nb - sb * SB)
        for bl in range(nb_sb):
            if sum(tiles_shb[sb][h][bl] for h in range(nch)) == 0:
                tiles_shb[sb][0][bl] = 1

    sched = build_schedule(tiles_shb, nb, nch, rows_per_core)
    T = sched.T

    start_shb = np.zeros((nsb, nch, nb), np.int64)
    t = 0
    for sb in range(nsb):
        nb_sb = min(SB, nb - sb * SB)
        for h in range(nch):
            for bl in range(nb_sb):
                start_shb[sb, h, sb * SB + bl] = t * 128
                t += tiles_shb[sb][h][bl]

    idx16 = np.zeros((C, 128, 8 * T), np.int16)
    val = np.zeros((C, 128, T), np.float32)
    slot = np.zeros((C, 128, T), np.float32)
    for c in range(C):
        lr, cc, vv = per_core[c]
        b = lr // M
        h = chunk_of(cc)
        sbk = b // SB
        n = len(lr)
        if n:
            key = (sbk * nch + h) * nb + b
            order = np.argsort(key, kind="stable")
            lr, cc, vv, key = lr[order], cc[order], vv[order], key[order]
            grp_start = np.searchsorted(key, key)
            within = np.arange(n) - grp_start
            pos = start_shb.reshape(-1)[key] + within
            ti = pos // 128
            j = pos % 128
            cloc = local_of(cc).astype(np.int16)
            col16 = 8 * ti + j // 16
            row16 = j % 16
            a16 = np.zeros((16, 8 * T), np.int16)
            a16[row16, col16] = cloc
            idx16[c] = np.tile(a16, (8, 1))
            val[c][j, ti] = vv.astype(np.float32)
            slot[c][j, ti] = (lr - b * M).astype(np.float32)
    return sched, idx16, val, slot


# ---------------------------------------------------------------- program

def build_program(V, E, n_cores, s1: SpmmSchedule, s2: SpmmSchedule):
    nc = bacc.Bacc(
        "TRN2", target_bir_lowering=False, debug=False, num_devices=n_cores,
        dynamic_dma_scratch_size=SCRATCH, num_swdge_queues=NQ,
    )
    nb1 = len(s1.blocks)
    nb2 = len(s2.blocks)

    embs = nc.dram_tensor("embs", [V, D], F32, kind="ExternalInput")
    g1_idx = nc.dram_tensor("g1_idx", [128, 8 * s1.T], I16, kind="ExternalInput")
    g1_val = nc.dram_tensor("g1_val", [128, s1.T], F32, kind="ExternalInput")
    g1_slot = nc.dram_tensor("g1_slot", [128, s1.T], F32, kind="ExternalInput")
    g2_idx = nc.dram_tensor("g2_idx", [128, 8 * s2.T], I16, kind="ExternalInput")
    g2_val = nc.dram_tensor("g2_val", [128, s2.T], F32, kind="ExternalInput")
    g2_slot = nc.dram_tensor("g2_slot", [128, s2.T], F32, kind="ExternalInput")
    mask_u8 = nc.dram_tensor(
        "mask_u8", [M, nb2 * D], mybir.dt.uint8, kind="ExternalInput"
    )
    iota_in = nc.dram_tensor("iota_in", [128, M], F32, kind="ExternalInput")
    y = nc.dram_tensor("y", [M, nb2, D], F32, kind="ExternalOutput")

    msg_part = nc.dram_tensor("msg_part", [M, nb1, D], F32)
    msg_rows = n_cores * M * nb1
    msg_full = nc.dram_tensor("msg_full", [msg_rows, D], F32, addr_space="Shared")

    nch1 = -(-V // CHUNK)
    tables1 = [
        (embs, h * CHUNK, min(CHUNK, V - h * CHUNK)) for h in range(nch1)
    ]
    nch2 = -(-msg_rows // CHUNK)
    tables2 = [
        (msg_full, h * CHUNK, min(CHUNK, msg_rows - h * CHUNK))
        for h in range(nch2)
    ]

    with tile.TileContext(nc) as tc:
        with tc.tile_pool(name="const", bufs=1) as constp:
            iota_sb = constp.tile([128, M], F32, tag="iota")
            nc.sync.dma_start(iota_sb[:], iota_in[:])
            qstate = [0]

            def emit_spmm(tag, sched, gi, gv, gs, tables, sb_writer,
                          streams, idxseg, gpool, spool, psum_pool):
                v_sb = streams.tile([128, sched.T], F32, tag=f"{tag}v")
                s_sb = streams.tile([128, sched.T], F32, tag=f"{tag}s")
                nc.sync.dma_start(v_sb[:], gv[:])
                nc.sync.dma_start(s_sb[:], gs[:])

                sb_t0, sb_t1 = [], []
                for sb in range(sched.nsb):
                    ts = [c[2] for c in sched.calls if c[0] == sb]
                    te = [c[2] + c[3] for c in sched.calls if c[0] == sb]
                    sb_t0.append(min(ts))
                    sb_t1.append(max(te))
                max_seg = max(b - a for a, b in zip(sb_t0, sb_t1))

                blocks = sched.blocks
                call_i = 0
                for sb in range(sched.nsb):
                    t0s, t1s = sb_t0[sb], sb_t1[sb]
                    iseg = idxseg.tile([128, 8 * max_seg], I16, tag=f"{tag}iseg")
                    nc.sync.dma_start(
                        iseg[:, : 8 * (t1s - t0s)], gi[:, 8 * t0s : 8 * t1s]
                    )
                    banks = []
                    for _j in range(sched.sb_ntiles_j[sb]):
                        bank_tile = psum_pool.tile(
                            [128, D], F32, tag="bank", name=f"{tag}bank{sb}_{_j}"
                        )
                        banks.append(bank_tile)
                    while call_i < len(sched.calls) and sched.calls[call_i][0] == sb:
                        _, h, t0, w = sched.calls[call_i]
                        call_i += 1
                        tab, tab_base, tab_rows = tables[h]
                        gbuf = gpool.tile([128, W, D], F32, tag="g")
                        nc.gpsimd.dma_gather(
                            out_ap=gbuf[:, :w, :],
                            in_ap=tab[tab_base : tab_base + tab_rows, :],
                            idxs_ap=iseg[:, 8 * (t0 - t0s) : 8 * (t0 - t0s + w)],
                            num_idxs=w * 128,
                            num_idxs_reg=w * 128,
                            elem_size=D,
                            queue_num=qstate[0],
                        )
                        qstate[0] = (qstate[0] + 1) % NQ
                        S = spool.tile([128, W, M], F32, tag="S")
                        slot_b = (
                            s_sb[:, t0 : t0 + w].unsqueeze(2).to_broadcast([128, w, M])
                        )
                        iota_b = iota_sb[:].unsqueeze(1).to_broadcast([128, w, M])
                        nc.vector.tensor_tensor(
                            S[:, :w, :], slot_b, iota_b, mybir.AluOpType.is_equal
                        )
                        val_b = (
                            v_sb[:, t0 : t0 + w].unsqueeze(2).to_broadcast([128, w, M])
                        )
                        nc.vector.tensor_tensor(
                            S[:, :w, :], S[:, :w, :], val_b, mybir.AluOpType.mult
                        )
                        for j in range(w):
                            t = t0 + j
                            bi = blocks[sched.tile_block[t]]
                            nc.tensor.matmul(
                                banks[bi.tile_j][M * bi.half : M * (bi.half + 1), :],
                                S[:, j, :],
                                gbuf[:, j, :],
                                start=(t == bi.first_t),
                                stop=(t == bi.last_t),
                            )
                    sb_writer(sb, min(SB, len(blocks) - sb * SB), banks)

            # ---------------- SpMM1
            with (
                tc.tile_pool(name="s1streams", bufs=1) as streams1,
                tc.tile_pool(name="s1idxseg", bufs=2) as idxseg1,
                tc.tile_pool(name="gpool1", bufs=6) as gpool1,
                tc.tile_pool(name="spool1", bufs=6) as spool1,
                tc.tile_pool(name="psum1", bufs=8, space="PSUM") as psum1,
                tc.tile_pool(name="stage1", bufs=3) as stage1,
            ):
                blocks1 = s1.blocks

                def sb_writer1(sb, nb_sb, banks):
                    wide = stage1.tile([M, SB, D], F32, tag="st1")
                    for bl in range(nb_sb):
                        bi = blocks1[sb * SB + bl]
                        nc.scalar.copy(
                            wide[:, bl, :],
                            banks[bi.tile_j][M * bi.half : M * (bi.half + 1), :],
                        )
                    nc.sync.dma_start(
                        msg_part[:, sb * SB : sb * SB + nb_sb, :],
                        wide[:, :nb_sb, :],
                    )

                emit_spmm("s1", s1, g1_idx, g1_val, g1_slot, tables1,
                          sb_writer1, streams1, idxseg1, gpool1, spool1, psum1)

            nc.gpsimd.collective_compute(
                "AllGather", mybir.AluOpType.bypass,
                replica_groups=[list(range(n_cores))],
                ins=[msg_part[:].opt()], outs=[msg_full[:].opt()],
            )

            # ---------------- SpMM2 (+ relu + dropout mask)
            with (
                tc.tile_pool(name="s2streams", bufs=1) as streams2,
                tc.tile_pool(name="s2idxseg", bufs=2) as idxseg2,
                tc.tile_pool(name="gpool2", bufs=6) as gpool2,
                tc.tile_pool(name="spool2", bufs=6) as spool2,
                tc.tile_pool(name="psum2", bufs=8, space="PSUM") as psum2,
                tc.tile_pool(name="stage2", bufs=3) as stage2,
                tc.tile_pool(name="maskp", bufs=1) as maskp,
            ):
                mask_sb = maskp.tile([M, nb2 * D], mybir.dt.uint8, tag="mk")
                nc.sync.dma_start(mask_sb[:], mask_u8[:])
                blocks2 = s2.blocks

                def sb_writer2(sb, nb_sb, banks):
                    wide = stage2.tile([M, SB, D], F32, tag="st2")
                    for bl in range(nb_sb):
                        b = sb * SB + bl
                        bi = blocks2[b]
                        nc.scalar.activation(
                            wide[:, bl, :],
                            banks[bi.tile_j][M * bi.half : M * (bi.half + 1), :],
                            mybir.ActivationFunctionType.Relu,
                        )
                        nc.vector.tensor_tensor(
                            wide[:, bl, :], wide[:, bl, :],
                            mask_sb[:, b * D : (b + 1) * D],
                            mybir.AluOpType.mult,
                        )
                    nc.sync.dma_start(
                        y[:, sb * SB : sb * SB + nb_sb, :], wide[:, :nb_sb, :]
                    )

                emit_spmm("s2", s2, g2_idx, g2_val, g2_slot, tables2,
                          sb_writer2, streams2, idxseg2, gpool2, spool2, psum2)

    nc.compile()
    return nc


# ---------------------------------------------------------------- driver

_CACHED = {"key": None, "nc": None}


def _dropout_keep_mask():
    """Reproduce jax.random.bernoulli(key(42), 0.7, (N_POIS, D)) on CPU."""
    import jax

    cpu = jax.devices("cpu")[0]
    with jax.default_device(cpu):
        keep = jax.random.bernoulli(
            jax.random.key(42), 1.0 - DROP_P, (N_POIS, D)
        )
        return np.asarray(keep)


def kernel(pois_embs, tar_rows, tar_cols, tar_vals, src_rows, src_cols,
           src_vals):
    V, E, C = N_POIS, N_EDGES, N_CORES
    pois_embs = np.ascontiguousarray(np.asarray(pois_embs, np.float32))
    assert pois_embs.shape == (V, D)
    inputs = {
        "tar_rows": tar_rows, "tar_cols": tar_cols, "tar_vals": tar_vals,
        "src_rows": src_rows, "src_cols": src_cols, "src_vals": src_vals,
    }
    keep = _dropout_keep_mask()
    drop_scale = 1.0 / (1.0 - DROP_P)
    rows1 = E // C
    nb1 = -(-rows1 // M)
    rows2 = V // C
    nb2 = -(-rows2 // M)

    nch1 = -(-V // CHUNK)
    s1, i1, v1, sl1 = prep_spmm(
        inputs["tar_rows"], inputs["tar_cols"], inputs["tar_vals"], E, C, nch1,
        chunk_of=lambda cc: cc // CHUNK,
        local_of=lambda cc: cc - (cc // CHUNK) * CHUNK,
    )

    def pid_of(cc):
        owner = cc // rows1
        off = cc % rows1
        return (owner * M + off % M) * nb1 + off // M

    msg_rows = C * M * nb1
    nch2 = -(-msg_rows // CHUNK)
    s2, i2, v2, sl2 = prep_spmm(
        inputs["src_rows"], inputs["src_cols"],
        np.asarray(inputs["src_vals"], np.float32) * np.float32(drop_scale),
        V, C, nch2,
        chunk_of=lambda cc: pid_of(cc) // CHUNK,
        local_of=lambda cc: pid_of(cc) - (pid_of(cc) // CHUNK) * CHUNK,
    )

    iota = np.ascontiguousarray(
        np.broadcast_to(np.arange(M, dtype=np.float32), (128, M))
    )
    in_maps = []
    for c in range(C):
        km = keep[c * rows2 : (c + 1) * rows2]
        mk = np.zeros((M, nb2 * D), np.uint8)
        for b in range(nb2):
            prows = min(M, rows2 - b * M)
            mk[:prows, b * D : b * D + D] = km[b * M : b * M + prows]
        in_maps.append(
            {
                "embs": pois_embs,
                "g1_idx": i1[c], "g1_val": v1[c], "g1_slot": sl1[c],
                "g2_idx": i2[c], "g2_val": v2[c], "g2_slot": sl2[c],
                "mask_u8": mk,
                "iota_in": iota,
            }
        )

    key = (tuple(s1.calls), tuple(s2.calls))
    if _CACHED["key"] != key:
        _CACHED["nc"] = build_program(V, E, C, s1, s2)
        _CACHED["key"] = key
    nc = _CACHED["nc"]

    last_err = None
    for _ in range(3):
        try:
            res = run_bass_kernel_spmd(
                nc, in_maps, core_ids=list(range(C)), trace=False
            )
            break
        except Exception as e:  # transient device wedges recover on retry
            last_err = e
            import time as _time

            _time.sleep(30)
    else:
        raise last_err

    outs = []
    for c in range(C):
        ya = res.results[c]["y"]  # [M, nb2, D]
        outs.append(ya.transpose(1, 0, 2).reshape(nb2 * M, D)[:rows2])
    return np.ascontiguousarray(np.concatenate(outs, axis=0))
